# revision 9
# baseline (speedup 1.0000x reference)
"""Trainium2 Bass kernel for dilated local attention (nn_DilateAttention).

Problem: x [8, 64, 64, 256] f32, W_qkv [768, 256] f32.
  qkv = x @ W_qkv.T; per pixel, per head (8 heads x 32 dim): attention over
  the 9 dilated (3x3, dilation 3) spatial neighbors with zero padding.

Design (one image per core, data-parallel over batch):
  - Fully fused software pipeline: x arrives in 4 casting DMA pieces
    (f32 -> bf16, gpsimd SWDGE), each piece is PE-transposed (bf16, 1
    cyc/row) and projected (bf16 matmuls, W^T stationary) into q/k/v in
    [c, (j, m)] layout with zero borders; attention chunks of 1024 pixels
    are emitted as soon as the k/v region they touch is ready, so every
    engine's in-order stream matches the dataflow.
  - Scores: DVE products q*k_shift on [128, 2, 1024] tiles (2x_1p mode),
    reduced per-head by tiny-output PE matmuls (stationary = product tile,
    moving = ones_s [128, 4]; cost = 4 output rows each).
  - Softmax in [m, 72] layout, 512-pixel groups packed [128, 4, 72] in one
    PSUM bank; single broadcast [128, 72] mask tile; invalid (column-wrap)
    slots are masked and the denominator corrected by the invalid count
    (reference zero-pads keys -> exp(0)=1 per invalid slot); normalization
    folded in before the PE transpose to [72, m]; mask/normalize
    multiplies on GPSIMD (SBUF only - GPSIMD cannot touch PSUM).
  - AV: PE broadcast matmuls (bkk stationary, attn72 moving) -> f32 PSUM
    pairs, one wide ACT evac each -> bf16 SBUF -> DVE multiply with
    shifted v at 2x -> 9-tap TRANSPOSED accumulation: t2 slices are the
    stationary and identb the moving operand, so the accumulated output
    lands directly in [m, c] layout (no output transposes, no extra evac).
  - Output stored with 16 small DMAs as [m, c] regions complete.
"""

import sys

sys.path.insert(0, "/opt/trn_rl_repo")

import numpy as np
import ml_dtypes
from contextlib import ExitStack

import concourse.bass as bass
import concourse.bacc as bacc
import concourse.tile as tile
from concourse import mybir
from concourse.bass_utils import run_bass_kernel_spmd

B, H, W, C = 8, 64, 64, 256
NH, DPH, K2 = 8, 32, 9
N = H * W          # 4096 pixels
PAD = 256          # zero border on each side of k/v (covers |delta| <= 195)
SCALE = DPH ** -0.5
F32 = mybir.dt.float32
F32R = mybir.dt.float32r
BF16 = mybir.dt.bfloat16
NPBF16 = ml_dtypes.bfloat16

DELTAS = [64 * (3 * i - 3) + (3 * j - 3) for i in range(3) for j in range(3)]

CH = 1024                     # attention chunk width
NCH = N // CH
S5 = CH // 512                # 512-groups per chunk


def _host_consts():
    ident = np.eye(128, dtype=np.float32)
    identb = np.eye(128, dtype=NPBF16)
    # score reduce (moving operand): ones_s[p, nn] = 1 iff p//32 == nn
    ones_s = np.zeros((128, 4), NPBF16)
    for p in range(128):
        ones_s[p, p // 32] = 1.0
    # attn broadcast: B[p, j, kk, q] = 1 iff p == (4j + q//32)*9 + kk
    bkk = np.zeros((72, 2, 9, 128), NPBF16)
    for jj in range(2):
        for kk in range(9):
            for q in range(128):
                bkk[(4 * jj + q // 32) * 9 + kk, jj, kk, q] = 1.0
    bkk = bkk.reshape(72, 2 * 9 * 128)
    # column-validity mask in [p, h*9+kk] layout (identical for every
    # 128-pixel group since p % 64 == m % 64), plus invalid counts
    maskT = np.zeros((128, 72), NPBF16)
    cntT = np.zeros((128, 1), np.float32)
    jm = np.arange(128) % 64
    for kk in range(9):
        dc = 3 * (kk % 3) - 3
        valid = (((jm + dc) >= 0) & ((jm + dc) < 64)).astype(np.float32)
        for h in range(8):
            maskT[:, h * 9 + kk] = valid
        cntT[:, 0] += 1.0 - valid
    return ident, identb, ones_s, bkk, maskT, cntT


def build_nc() -> bass.Bass:
    nc = bacc.Bacc()
    x_d = nc.declare_dram_parameter("x", [N, C], F32, isOutput=False)
    w_d = nc.declare_dram_parameter("w", [3 * C, C], F32, isOutput=False)
    ident_d = nc.declare_dram_parameter("ident", [128, 128], F32, isOutput=False)
    identb_d = nc.declare_dram_parameter("identb", [128, 128], BF16, isOutput=False)
    ones_s_d = nc.declare_dram_parameter("ones_s", [128, 4], BF16, isOutput=False)
    bkk_d = nc.declare_dram_parameter("bkk", [72, 2 * 9 * 128], BF16, isOutput=False)
    maskT_d = nc.declare_dram_parameter("maskT", [128, 72], BF16, isOutput=False)
    cntT_d = nc.declare_dram_parameter("cntT", [128, 1], F32, isOutput=False)
    out_d = nc.declare_dram_parameter("out", [N, C], F32, isOutput=True)

    with tile.TileContext(nc) as tc, ExitStack() as ctx:
        singles = ctx.enter_context(tc.tile_pool(name="singles", bufs=1))
        qkv_pool = ctx.enter_context(tc.tile_pool(name="qkv", bufs=1))

        ident = singles.tile([128, 128], F32)
        nc.scalar.dma_start(out=ident, in_=ident_d[:, :])

        # q/k/v in transposed [c, (j, m)] bf16 layout; k/v have zero borders
        qT = qkv_pool.tile([128, 2, N], BF16, name="qT")
        kT = qkv_pool.tile([128, 2, N + 2 * PAD], BF16, name="kT")
        vT = qkv_pool.tile([128, 2, N + 2 * PAD], BF16, name="vT")
        nc.gpsimd.memset(kT[:, :, 0:PAD], 0.0)
        nc.gpsimd.memset(kT[:, :, PAD + N:], 0.0)
        nc.gpsimd.memset(vT[:, :, 0:PAD], 0.0)
        nc.gpsimd.memset(vT[:, :, PAD + N:], 0.0)
        attn72 = qkv_pool.tile([72, N], BF16, name="attn72")

        # ---- x in 4 DMA pieces (f32r), W + consts interleaved ----
        xt_pool = tc.alloc_tile_pool(name="xt_pool", bufs=1)
        ctx.callback(xt_pool.release)
        stage_pool = tc.alloc_tile_pool(name="stage", bufs=1)
        ctx.callback(stage_pool.release)
        win = stage_pool.tile([128, 6, 256], F32)
        xin_pool = tc.alloc_tile_pool(name="xin_pool", bufs=2)
        ctx.callback(xin_pool.release)
        xv = x_d[:, :].rearrange("(t p) c -> p t c", p=128)
        piece_tiles = {}

        def load_piece(piece):
            t = xin_pool.tile([128, 8, 256], BF16, name="xin")
            piece_tiles[piece] = t
            nc.gpsimd.dma_start(
                out=t, in_=xv[:, piece * 8:(piece + 1) * 8, :])

        nc.sync.dma_start(
            out=win, in_=w_d[:, :].rearrange("(t p) c -> p t c", p=128))
        load_piece(0)
        load_piece(1)
        ones_s = singles.tile([128, 4], BF16)
        nc.scalar.dma_start(out=ones_s, in_=ones_s_d[:, :])
        load_piece(2)
        maskT = singles.tile([128, 72], BF16)
        nc.scalar.dma_start(out=maskT, in_=maskT_d[:, :])
        cntT = singles.tile([128, 1], F32)
        nc.scalar.dma_start(out=cntT, in_=cntT_d[:, :])
        identb = singles.tile([128, 128], BF16)
        nc.scalar.dma_start(out=identb, in_=identb_d[:, :])
        load_piece(3)
        bkk = singles.tile([72, 2, 9, 128], BF16)
        nc.scalar.dma_start(
            out=bkk, in_=bkk_d[:, :].rearrange("p (j k q) -> p j k q", j=2, k=9))

        xT = [xt_pool.tile([128, N], BF16, name=f"xT{j}") for j in range(2)]
        wlhsT = [singles.tile([128, 6, 128], BF16, name=f"wlhsT{j}")
                 for j in range(2)]
        out_view = out_d[:, :].rearrange("(t p) (j c) -> p t j c", p=128, j=2)

        with tc.tile_pool(name="ps512", bufs=2, space="PSUM") as accp, \
             tc.tile_pool(name="t_sb", bufs=3) as tsb, \
             tc.tile_pool(name="sm_sb", bufs=5) as smb, \
             tc.tile_pool(name="bcs_sb", bufs=3) as bsb, \
             tc.tile_pool(name="t2_sb", bufs=11) as t2b, \
             tc.tile_pool(name="of_sb", bufs=2) as ofb, \
             tc.tile_pool(name="s4_ps", bufs=2, space="PSUM") as sps, \
             tc.tile_pool(name="bc_ps", bufs=2, space="PSUM") as bcp:

            def trans_mt2(mt2, evac_eng):
                xt_ps = accp.tile([128, 512], F32, name="w512").bitcast(
                    BF16)[:, 0:512].rearrange("p (a c) -> p a c", a=2)
                for mi in range(2):
                    mt = mt2 * 2 + mi
                    xin_t = piece_tiles[mt // 8]
                    for j in range(2):
                        nc.tensor.transpose(
                            xt_ps[:, mi, j * 128:(j + 1) * 128],
                            xin_t[:, mt % 8, j * 128:(j + 1) * 128], identb)
                for j in range(2):
                    dst = xT[j].rearrange("p (t q) -> p t q", q=128)[
                        :, mt2 * 2:mt2 * 2 + 2, :]
                    src = xt_ps[:, :, j * 128:(j + 1) * 128]
                    nc.vector.tensor_copy(out=dst, in_=src)

            def w_transposes():
                for ot in range(0, 6, 2):
                    wt_ps = accp.tile([128, 512], F32, name="w512")
                    wtv = wt_ps.rearrange("p (b q) -> p b q", q=128)
                    for o2 in range(2):
                        for j in range(2):
                            nc.tensor.transpose(
                                wtv[:, o2 * 2 + j, :],
                                win[:, ot + o2, j * 128:(j + 1) * 128], ident)
                    for j in range(2):
                        nc.scalar.copy(
                            out=wlhsT[j][:, ot:ot + 2, :],
                            in_=wtv.rearrange("p (o2 j) q -> p o2 j q", o2=2)[
                                :, :, j, :])

            def proj_ch(ch):
                for ot in (2, 3, 0, 1, 4, 5):
                    acc = accp.tile([128, 512], F32, name="w512")
                    for j in range(2):
                        nc.tensor.matmul(
                            acc, wlhsT[j][:, ot, :],
                            xT[j][:, ch * 512:(ch + 1) * 512],
                            start=(j == 0), stop=(j == 1))
                    dst_j = ot % 2
                    if ot < 2:
                        dst = qT[:, dst_j, ch * 512:(ch + 1) * 512]
                    elif ot < 4:
                        dst = kT[:, dst_j, PAD + ch * 512:PAD + (ch + 1) * 512]
                    else:
                        dst = vT[:, dst_j, PAD + ch * 512:PAD + (ch + 1) * 512]
                    if ot < 4:
                        nc.scalar.copy(out=dst, in_=acc)
                    else:
                        nc.vector.tensor_copy(out=dst, in_=acc)

            s4_of = {}

            def attn_scores(c4):
                # --- scores: products + tiny-out reductions ---
                s4 = [sps.tile([128, 4, 72], F32, name="s4") for _ in range(S5)]
                s4_of[c4] = s4
                for kk in range(K2):
                    dl = DELTAS[kk]
                    t_t = tsb.tile([128, 2, CH], BF16, name="t_t")
                    peng = nc.gpsimd if kk == 5 else nc.vector
                    peng.tensor_mul(
                        t_t, qT[:, :, c4 * CH:(c4 + 1) * CH],
                        kT[:, :, PAD + c4 * CH + dl:PAD + (c4 + 1) * CH + dl])
                    for j in range(2):
                        for s5 in range(S5):
                            for su in range(4):
                                out_ap = s4[s5].rearrange(
                                    "p s (h k) -> p s h k", k=9)[
                                    :, su, 4 * j:4 * j + 4, kk]
                                nc.tensor.matmul(
                                    out_ap,
                                    t_t[:, j, s5 * 512 + su * 128:
                                        s5 * 512 + (su + 1) * 128],
                                    ones_s, start=True, stop=True)
            def attn_softmax(c4):
                s4 = s4_of.pop(c4)
                # --- softmax + transpose to attn72 ---
                at_ps = bcp.tile([128, CH], F32, name="bcp").bitcast(
                    BF16)[0:72, 0:CH].rearrange("p (s q) -> p s q", q=128)
                for s5 in range(S5):
                    em4 = smb.tile([128, 4, 72], BF16, name="em4")
                    nc.scalar.activation(
                        em4, s4[s5], mybir.ActivationFunctionType.Exp,
                        scale=float(SCALE))
                    em4m = smb.tile([128, 4, 72], BF16, name="em4m")
                    nc.gpsimd.tensor_mul(
                        em4m, em4,
                        maskT[:, None, :].broadcast_to((128, 4, 72)))
                    den4 = smb.tile([128, 4, 8], F32, name="den4")
                    nc.vector.reduce_sum(
                        den4, em4m.rearrange("p s (h k) -> p s h k", k=9),
                        axis=mybir.AxisListType.X)
                    nc.vector.tensor_scalar_add(
                        out=den4.rearrange("p s h -> p (s h)"),
                        in0=den4.rearrange("p s h -> p (s h)"),
                        scalar1=cntT[:, 0:1])
                    rr4 = smb.tile([128, 4, 8], F32, name="rr4")
                    nc.vector.reciprocal(rr4, den4)
                    em4n = smb.tile([128, 4, 72], BF16, name="em4n")
                    nc.gpsimd.tensor_mul(
                        em4n.rearrange("p s (h k) -> p s h k", k=9),
                        em4m.rearrange("p s (h k) -> p s h k", k=9),
                        rr4[:, :, :, None].broadcast_to((128, 4, 8, 9)))
                    for su in range(4):
                        nc.tensor.transpose(
                            at_ps[:, s5 * 4 + su, :], em4n[:, su, :], identb)
                nc.vector.tensor_copy(
                    out=attn72[:, c4 * CH:(c4 + 1) * CH],
                    in_=at_ps.rearrange("p s q -> p (s q)"))

            def attn_av(c4):
                # --- AV for this chunk ---
                t2s = []
                for kk in range(K2):
                    dl = DELTAS[kk]
                    bcs = bsb.tile([128, 2, CH], BF16, name="bcs")
                    for j in range(2):
                        bc_ps = bcp.tile([128, CH], F32, name="bcp")
                        for hf in range(S5):
                            nc.tensor.matmul(
                                bc_ps[:, hf * 512:(hf + 1) * 512],
                                bkk[:, j, kk, :],
                                attn72[:, c4 * CH + hf * 512:
                                       c4 * CH + (hf + 1) * 512],
                                start=True, stop=True)
                        nc.scalar.copy(out=bcs[:, j, :], in_=bc_ps)
                    t2 = t2b.tile([128, 2, CH], BF16, name="t2")
                    if kk in (3, 5, 7):
                        nc.gpsimd.tensor_mul(
                            t2, bcs,
                            vT[:, :, PAD + c4 * CH + dl:PAD + (c4 + 1) * CH + dl])
                    else:
                        nc.vector.tensor_mul(
                            t2, bcs,
                            vT[:, :, PAD + c4 * CH + dl:PAD + (c4 + 1) * CH + dl])
                    t2s.append(t2)
                for s5 in range(S5):
                    for j in range(2):
                        ch = c4 * S5 + s5
                        # transposed accumulation: t2 slices stationary,
                        # identb moving -> out lands in [m, c] layout
                        o_ps = accp.tile([128, 512], F32, name="w512")
                        otv = o_ps.rearrange("p (t q) -> p t q", q=128)
                        for su in range(4):
                            base = s5 * 512 + su * 128
                            for kk in range(K2):
                                nc.tensor.matmul(
                                    otv[:, su, :],
                                    t2s[kk][:, j, base:base + 128],
                                    identb,
                                    start=(kk == 0), stop=(kk == K2 - 1))
                        o_fin = ofb.tile([128, 4, 128], F32, name="o_fin")
                        if c4 == NCH - 1:
                            nc.scalar.copy(out=o_fin, in_=otv)
                        else:
                            nc.vector.tensor_copy(out=o_fin, in_=otv)
                        nc.sync.dma_start(
                            out=out_view[:, ch * 4:(ch + 1) * 4, j, :],
                            in_=o_fin)

            # ---- fused pipeline ----
            for mt2 in range(0, 4):
                trans_mt2(mt2, "vector")
            w_transposes()
            proj_ch(0)
            proj_ch(1)
            for mt2 in range(4, 8):
                trans_mt2(mt2, "vector")
            proj_ch(2)
            proj_ch(3)
            attn_scores(0)
            for mt2 in range(8, 12):
                trans_mt2(mt2, "scalar")
            proj_ch(4)
            proj_ch(5)
            for mt2 in range(12, 16):
                trans_mt2(mt2, "scalar")
            proj_ch(6)
            proj_ch(7)
            for c4 in range(NCH):
                attn_softmax(c4)
                if c4 + 1 < NCH:
                    attn_scores(c4 + 1)
                attn_av(c4)
    nc.compile()
    return nc


_NC_CACHE = None


def kernel(x: np.ndarray, W_qkv: np.ndarray) -> np.ndarray:
    global _NC_CACHE
    if _NC_CACHE is None:
        _NC_CACHE = build_nc()
    nc = _NC_CACHE

    x = np.ascontiguousarray(x, dtype=np.float32)
    W_qkv = np.ascontiguousarray(W_qkv, dtype=np.float32)
    ident, identb, ones_s, bkk, maskT, cntT = _host_consts()
    consts = {
        "w": W_qkv, "ident": ident, "identb": identb, "ones_s": ones_s,
        "bkk": bkk, "maskT": maskT, "cntT": cntT,
    }
    in_maps = [
        {"x": x[b].reshape(N, C).copy(), **consts} for b in range(B)
    ]
    res = run_bass_kernel_spmd(nc, in_maps, list(range(B)))
    out = np.stack([res.results[b]["out"].reshape(H, W, C) for b in range(B)])
    return out


if __name__ == "__main__":
    rng = np.random.default_rng(0)
    x = rng.standard_normal((B, H, W, C), dtype=np.float32)
    wq = (rng.standard_normal((3 * C, C), dtype=np.float32) * 0.02).astype(np.float32)
    out = kernel(x, wq)
    print("out", out.shape, out.dtype, float(np.abs(out).mean()))


# revision 10
# speedup vs baseline: 1.0061x; 1.0061x over previous
"""Trainium2 Bass kernel for dilated local attention (nn_DilateAttention).

Problem: x [8, 64, 64, 256] f32, W_qkv [768, 256] f32.
  qkv = x @ W_qkv.T; per pixel, per head (8 heads x 32 dim): attention over
  the 9 dilated (3x3, dilation 3) spatial neighbors with zero padding.

Design (one image per core, data-parallel over batch):
  - Fully fused software pipeline: x arrives in 4 casting DMA pieces
    (f32 -> bf16, gpsimd SWDGE), each piece is PE-transposed (bf16, 1
    cyc/row) and projected (bf16 matmuls, W^T stationary) into q/k/v in
    [c, (j, m)] layout with zero borders; attention chunks of 1024 pixels
    are emitted as soon as the k/v region they touch is ready, so every
    engine's in-order stream matches the dataflow.
  - Scores: DVE products q*k_shift on [128, 2, 1024] tiles (2x_1p mode),
    reduced per-head by tiny-output PE matmuls (stationary = product tile,
    moving = ones_s [128, 4]; cost = 4 output rows each).
  - Softmax in [m, 72] layout, 512-pixel groups packed [128, 4, 72] in one
    PSUM bank; single broadcast [128, 72] mask tile; invalid (column-wrap)
    slots are masked and the denominator corrected by the invalid count
    (reference zero-pads keys -> exp(0)=1 per invalid slot); normalization
    folded in before the PE transpose to [72, m]; mask/normalize
    multiplies on GPSIMD (SBUF only - GPSIMD cannot touch PSUM).
  - AV: PE broadcast matmuls (bkk stationary, attn72 moving) -> f32 PSUM
    pairs, one wide ACT evac each -> bf16 SBUF -> DVE multiply with
    shifted v at 2x -> 9-tap TRANSPOSED accumulation: t2 slices are the
    stationary and identb the moving operand, so the accumulated output
    lands directly in [m, c] layout (no output transposes, no extra evac).
  - Output stored with 16 small DMAs as [m, c] regions complete.
"""

import sys

sys.path.insert(0, "/opt/trn_rl_repo")

import numpy as np
import ml_dtypes
from contextlib import ExitStack

import concourse.bass as bass
import concourse.bacc as bacc
import concourse.tile as tile
from concourse import mybir
from concourse.bass_utils import run_bass_kernel_spmd

B, H, W, C = 8, 64, 64, 256
NH, DPH, K2 = 8, 32, 9
N = H * W          # 4096 pixels
PAD = 256          # zero border on each side of k/v (covers |delta| <= 195)
SCALE = DPH ** -0.5
F32 = mybir.dt.float32
F32R = mybir.dt.float32r
BF16 = mybir.dt.bfloat16
NPBF16 = ml_dtypes.bfloat16

DELTAS = [64 * (3 * i - 3) + (3 * j - 3) for i in range(3) for j in range(3)]

CH = 1024                     # attention chunk width
NCH = N // CH
S5 = CH // 512                # 512-groups per chunk


def _host_consts():
    ident = np.eye(128, dtype=np.float32)
    identb = np.eye(128, dtype=NPBF16)
    # score reduce (moving operand): ones_s[p, nn] = 1 iff p//32 == nn
    ones_s = np.zeros((128, 4), NPBF16)
    for p in range(128):
        ones_s[p, p // 32] = 1.0
    # attn broadcast: B[p, j, kk, q] = 1 iff p == (4j + q//32)*9 + kk
    bkk = np.zeros((72, 2, 9, 128), NPBF16)
    for jj in range(2):
        for kk in range(9):
            for q in range(128):
                bkk[(4 * jj + q // 32) * 9 + kk, jj, kk, q] = 1.0
    bkk = bkk.reshape(72, 2 * 9 * 128)
    # column-validity mask in [p, h*9+kk] layout (identical for every
    # 128-pixel group since p % 64 == m % 64), plus invalid counts
    maskT = np.zeros((128, 72), NPBF16)
    cntT = np.zeros((128, 1), np.float32)
    jm = np.arange(128) % 64
    for kk in range(9):
        dc = 3 * (kk % 3) - 3
        valid = (((jm + dc) >= 0) & ((jm + dc) < 64)).astype(np.float32)
        for h in range(8):
            maskT[:, h * 9 + kk] = valid
        cntT[:, 0] += 1.0 - valid
    return ident, identb, ones_s, bkk, maskT, cntT


def build_nc() -> bass.Bass:
    nc = bacc.Bacc()
    x_d = nc.declare_dram_parameter("x", [N, C], F32, isOutput=False)
    w_d = nc.declare_dram_parameter("w", [3 * C, C], F32, isOutput=False)
    ident_d = nc.declare_dram_parameter("ident", [128, 128], F32, isOutput=False)
    identb_d = nc.declare_dram_parameter("identb", [128, 128], BF16, isOutput=False)
    ones_s_d = nc.declare_dram_parameter("ones_s", [128, 4], BF16, isOutput=False)
    bkk_d = nc.declare_dram_parameter("bkk", [72, 2 * 9 * 128], BF16, isOutput=False)
    maskT_d = nc.declare_dram_parameter("maskT", [128, 72], BF16, isOutput=False)
    cntT_d = nc.declare_dram_parameter("cntT", [128, 1], F32, isOutput=False)
    out_d = nc.declare_dram_parameter("out", [N, C], F32, isOutput=True)

    with tile.TileContext(nc) as tc, ExitStack() as ctx:
        singles = ctx.enter_context(tc.tile_pool(name="singles", bufs=1))
        qkv_pool = ctx.enter_context(tc.tile_pool(name="qkv", bufs=1))

        ident = singles.tile([128, 128], F32)
        nc.scalar.dma_start(out=ident, in_=ident_d[:, :])

        # q/k/v in transposed [c, (j, m)] bf16 layout; k/v have zero borders
        qT = qkv_pool.tile([128, 2, N], BF16, name="qT")
        kT = qkv_pool.tile([128, 2, N + 2 * PAD], BF16, name="kT")
        vT = qkv_pool.tile([128, 2, N + 2 * PAD], BF16, name="vT")
        nc.gpsimd.memset(kT[:, :, 0:PAD], 0.0)
        nc.gpsimd.memset(kT[:, :, PAD + N:], 0.0)
        nc.gpsimd.memset(vT[:, :, 0:PAD], 0.0)
        nc.gpsimd.memset(vT[:, :, PAD + N:], 0.0)
        attn72 = qkv_pool.tile([72, N], BF16, name="attn72")

        # ---- x in 4 DMA pieces (f32r), W + consts interleaved ----
        xt_pool = tc.alloc_tile_pool(name="xt_pool", bufs=1)
        ctx.callback(xt_pool.release)
        stage_pool = tc.alloc_tile_pool(name="stage", bufs=1)
        ctx.callback(stage_pool.release)
        win = stage_pool.tile([128, 6, 256], F32)
        xin_pool = tc.alloc_tile_pool(name="xin_pool", bufs=2)
        ctx.callback(xin_pool.release)
        xv = x_d[:, :].rearrange("(t p) c -> p t c", p=128)
        piece_tiles = {}

        def load_piece(piece):
            t = xin_pool.tile([128, 8, 256], BF16, name="xin")
            piece_tiles[piece] = t
            nc.gpsimd.dma_start(
                out=t, in_=xv[:, piece * 8:(piece + 1) * 8, :])

        nc.sync.dma_start(
            out=win, in_=w_d[:, :].rearrange("(t p) c -> p t c", p=128))
        load_piece(0)
        load_piece(1)
        ones_s = singles.tile([128, 4], BF16)
        nc.scalar.dma_start(out=ones_s, in_=ones_s_d[:, :])
        load_piece(2)
        maskT = singles.tile([128, 72], BF16)
        nc.scalar.dma_start(out=maskT, in_=maskT_d[:, :])
        cntT = singles.tile([128, 1], F32)
        nc.scalar.dma_start(out=cntT, in_=cntT_d[:, :])
        identb = singles.tile([128, 128], BF16)
        nc.scalar.dma_start(out=identb, in_=identb_d[:, :])
        load_piece(3)
        bkk = singles.tile([72, 2, 9, 128], BF16)
        nc.scalar.dma_start(
            out=bkk, in_=bkk_d[:, :].rearrange("p (j k q) -> p j k q", j=2, k=9))

        xT = [xt_pool.tile([128, N], BF16, name=f"xT{j}") for j in range(2)]
        wlhsT = [singles.tile([128, 6, 128], BF16, name=f"wlhsT{j}")
                 for j in range(2)]
        out_view = out_d[:, :].rearrange("(t p) (j c) -> p t j c", p=128, j=2)

        with tc.tile_pool(name="ps512", bufs=2, space="PSUM") as accp, \
             tc.tile_pool(name="t_sb", bufs=3) as tsb, \
             tc.tile_pool(name="sm_sb", bufs=5) as smb, \
             tc.tile_pool(name="bcs_sb", bufs=3) as bsb, \
             tc.tile_pool(name="t2_sb", bufs=11) as t2b, \
             tc.tile_pool(name="of_sb", bufs=2) as ofb, \
             tc.tile_pool(name="s4_ps", bufs=2, space="PSUM") as sps, \
             tc.tile_pool(name="bc_ps", bufs=2, space="PSUM") as bcp:

            def trans_mt2(mt2, evac_eng):
                xt_ps = accp.tile([128, 512], F32, name="w512").bitcast(
                    BF16)[:, 0:512].rearrange("p (a c) -> p a c", a=2)
                for mi in range(2):
                    mt = mt2 * 2 + mi
                    xin_t = piece_tiles[mt // 8]
                    for j in range(2):
                        nc.tensor.transpose(
                            xt_ps[:, mi, j * 128:(j + 1) * 128],
                            xin_t[:, mt % 8, j * 128:(j + 1) * 128], identb)
                for j in range(2):
                    dst = xT[j].rearrange("p (t q) -> p t q", q=128)[
                        :, mt2 * 2:mt2 * 2 + 2, :]
                    src = xt_ps[:, :, j * 128:(j + 1) * 128]
                    nc.vector.tensor_copy(out=dst, in_=src)

            def w_transposes():
                for ot in range(0, 6, 2):
                    wt_ps = accp.tile([128, 512], F32, name="w512")
                    wtv = wt_ps.rearrange("p (b q) -> p b q", q=128)
                    for o2 in range(2):
                        for j in range(2):
                            nc.tensor.transpose(
                                wtv[:, o2 * 2 + j, :],
                                win[:, ot + o2, j * 128:(j + 1) * 128], ident)
                    for j in range(2):
                        nc.scalar.copy(
                            out=wlhsT[j][:, ot:ot + 2, :],
                            in_=wtv.rearrange("p (o2 j) q -> p o2 j q", o2=2)[
                                :, :, j, :])

            def proj_ch(ch):
                for ot in (2, 3, 0, 1, 4, 5):
                    acc = accp.tile([128, 512], F32, name="w512")
                    for j in range(2):
                        nc.tensor.matmul(
                            acc, wlhsT[j][:, ot, :],
                            xT[j][:, ch * 512:(ch + 1) * 512],
                            start=(j == 0), stop=(j == 1))
                    dst_j = ot % 2
                    if ot < 2:
                        dst = qT[:, dst_j, ch * 512:(ch + 1) * 512]
                    elif ot < 4:
                        dst = kT[:, dst_j, PAD + ch * 512:PAD + (ch + 1) * 512]
                    else:
                        dst = vT[:, dst_j, PAD + ch * 512:PAD + (ch + 1) * 512]
                    if ot < 4:
                        nc.scalar.copy(out=dst, in_=acc)
                    else:
                        nc.vector.tensor_copy(out=dst, in_=acc)

            s4_of = {}

            def attn_scores(c4):
                # --- scores: products + tiny-out reductions ---
                s4 = [sps.tile([128, 4, 72], F32, name="s4") for _ in range(S5)]
                s4_of[c4] = s4
                for kk in range(K2):
                    dl = DELTAS[kk]
                    t_t = tsb.tile([128, 2, CH], BF16, name="t_t")
                    peng = nc.gpsimd if kk == 5 else nc.vector
                    peng.tensor_mul(
                        t_t, qT[:, :, c4 * CH:(c4 + 1) * CH],
                        kT[:, :, PAD + c4 * CH + dl:PAD + (c4 + 1) * CH + dl])
                    for j in range(2):
                        for s5 in range(S5):
                            for su in range(4):
                                out_ap = s4[s5].rearrange(
                                    "p s (h k) -> p s h k", k=9)[
                                    :, su, 4 * j:4 * j + 4, kk]
                                nc.tensor.matmul(
                                    out_ap,
                                    t_t[:, j, s5 * 512 + su * 128:
                                        s5 * 512 + (su + 1) * 128],
                                    ones_s, start=True, stop=True)
            def attn_softmax(c4):
                s4 = s4_of.pop(c4)
                # --- softmax + transpose to attn72 ---
                at_ps = bcp.tile([128, CH], F32, name="bcp").bitcast(
                    BF16)[0:72, 0:CH].rearrange("p (s q) -> p s q", q=128)
                for s5 in range(S5):
                    em4 = smb.tile([128, 4, 72], BF16, name="em4")
                    nc.scalar.activation(
                        em4, s4[s5], mybir.ActivationFunctionType.Exp,
                        scale=float(SCALE))
                    em4m = smb.tile([128, 4, 72], BF16, name="em4m")
                    nc.gpsimd.tensor_mul(
                        em4m, em4,
                        maskT[:, None, :].broadcast_to((128, 4, 72)))
                    den4 = smb.tile([128, 4, 8], F32, name="den4")
                    nc.vector.reduce_sum(
                        den4, em4m.rearrange("p s (h k) -> p s h k", k=9),
                        axis=mybir.AxisListType.X)
                    nc.vector.tensor_scalar_add(
                        out=den4.rearrange("p s h -> p (s h)"),
                        in0=den4.rearrange("p s h -> p (s h)"),
                        scalar1=cntT[:, 0:1])
                    rr4 = smb.tile([128, 4, 8], F32, name="rr4")
                    nc.vector.reciprocal(rr4, den4)
                    em4n = smb.tile([128, 4, 72], BF16, name="em4n")
                    nc.gpsimd.tensor_mul(
                        em4n.rearrange("p s (h k) -> p s h k", k=9),
                        em4m.rearrange("p s (h k) -> p s h k", k=9),
                        rr4[:, :, :, None].broadcast_to((128, 4, 8, 9)))
                    for su in range(4):
                        nc.tensor.transpose(
                            at_ps[:, s5 * 4 + su, :], em4n[:, su, :], identb)
                nc.vector.tensor_copy(
                    out=attn72[:, c4 * CH:(c4 + 1) * CH],
                    in_=at_ps.rearrange("p s q -> p (s q)"))

            def attn_av(c4):
                # --- AV for this chunk ---
                t2s = []
                for kk in range(K2):
                    dl = DELTAS[kk]
                    bcs = bsb.tile([128, 2, CH], BF16, name="bcs")
                    for j in range(2):
                        bc_ps = bcp.tile([128, CH], F32, name="bcp")
                        for hf in range(S5):
                            nc.tensor.matmul(
                                bc_ps[:, hf * 512:(hf + 1) * 512],
                                bkk[:, j, kk, :],
                                attn72[:, c4 * CH + hf * 512:
                                       c4 * CH + (hf + 1) * 512],
                                start=True, stop=True)
                        if kk == 5 and j == 1:
                            nc.vector.tensor_copy(out=bcs[:, j, :], in_=bc_ps)
                        else:
                            nc.scalar.copy(out=bcs[:, j, :], in_=bc_ps)
                    t2 = t2b.tile([128, 2, CH], BF16, name="t2")
                    if kk in (3, 5, 7):
                        nc.gpsimd.tensor_mul(
                            t2, bcs,
                            vT[:, :, PAD + c4 * CH + dl:PAD + (c4 + 1) * CH + dl])
                    else:
                        nc.vector.tensor_mul(
                            t2, bcs,
                            vT[:, :, PAD + c4 * CH + dl:PAD + (c4 + 1) * CH + dl])
                    t2s.append(t2)
                for s5 in range(S5):
                    for j in range(2):
                        ch = c4 * S5 + s5
                        # transposed accumulation: t2 slices stationary,
                        # identb moving -> out lands in [m, c] layout
                        o_ps = accp.tile([128, 512], F32, name="w512")
                        otv = o_ps.rearrange("p (t q) -> p t q", q=128)
                        for su in range(4):
                            base = s5 * 512 + su * 128
                            for kk in range(K2):
                                nc.tensor.matmul(
                                    otv[:, su, :],
                                    t2s[kk][:, j, base:base + 128],
                                    identb,
                                    start=(kk == 0), stop=(kk == K2 - 1))
                        o_fin = ofb.tile([128, 4, 128], F32, name="o_fin")
                        if c4 == NCH - 1:
                            nc.scalar.copy(out=o_fin, in_=otv)
                        else:
                            nc.vector.tensor_copy(out=o_fin, in_=otv)
                        nc.sync.dma_start(
                            out=out_view[:, ch * 4:(ch + 1) * 4, j, :],
                            in_=o_fin)

            # ---- fused pipeline ----
            for mt2 in range(0, 4):
                trans_mt2(mt2, "vector")
            w_transposes()
            proj_ch(0)
            proj_ch(1)
            for mt2 in range(4, 8):
                trans_mt2(mt2, "vector")
            proj_ch(2)
            proj_ch(3)
            attn_scores(0)
            for mt2 in range(8, 12):
                trans_mt2(mt2, "scalar")
            proj_ch(4)
            proj_ch(5)
            for mt2 in range(12, 16):
                trans_mt2(mt2, "scalar")
            proj_ch(6)
            proj_ch(7)
            for c4 in range(NCH):
                attn_softmax(c4)
                if c4 + 1 < NCH:
                    attn_scores(c4 + 1)
                attn_av(c4)
    nc.compile()
    return nc


_NC_CACHE = None


def kernel(x: np.ndarray, W_qkv: np.ndarray) -> np.ndarray:
    global _NC_CACHE
    if _NC_CACHE is None:
        _NC_CACHE = build_nc()
    nc = _NC_CACHE

    x = np.ascontiguousarray(x, dtype=np.float32)
    W_qkv = np.ascontiguousarray(W_qkv, dtype=np.float32)
    ident, identb, ones_s, bkk, maskT, cntT = _host_consts()
    consts = {
        "w": W_qkv, "ident": ident, "identb": identb, "ones_s": ones_s,
        "bkk": bkk, "maskT": maskT, "cntT": cntT,
    }
    in_maps = [
        {"x": x[b].reshape(N, C).copy(), **consts} for b in range(B)
    ]
    res = run_bass_kernel_spmd(nc, in_maps, list(range(B)))
    out = np.stack([res.results[b]["out"].reshape(H, W, C) for b in range(B)])
    return out


if __name__ == "__main__":
    rng = np.random.default_rng(0)
    x = rng.standard_normal((B, H, W, C), dtype=np.float32)
    wq = (rng.standard_normal((3 * C, C), dtype=np.float32) * 0.02).astype(np.float32)
    out = kernel(x, wq)
    print("out", out.shape, out.dtype, float(np.abs(out).mean()))
